# revision 10
# baseline (speedup 1.0000x reference)
"""BuzzLoss Trainium2 kernel.

Math (telescoped form of the reference):
    excl[t] = prod_{s<t} (1 - conf[s])          (exclusive cumprod)
    score_b = sum_t excl[b,t] * da[b,t]
    da[b,0] = acc[b,0];  da[b,t] = acc[b,t] - acc[b,t-1]
    out = -mean_b score_b

Derivation: buzz[t] = conf[t]*excl[t] = excl[t] - excl[t+1] telescopes, and
the correction term (1 - sum buzz) * acc[T-1] = cum[T-1]*acc[T-1] cancels
against the boundary of the summation-by-parts.

Sharding: pure data parallel — batch 8192 split across 8 NeuronCores (1024
rows each).  Each core emits per-row scores ([128 x 8] f32); the host sums,
takes the mean, and negates.  No collectives.

Per 128-row tile on-chip:
    ACT   : nb = 1 - conf                   (activation Copy, scale=-1, bias=1)
    GPSIMD: excl[:, 0] = 1                  (memset)
    DVE   : excl[:, 1:] = prefix scan        (tensor_tensor_scan, mult — the
            hardware recurrence computes the whole cumprod in one instruction)
    GPSIMD: da = shifted subtract of acc     (tensor_sub)
    DVE   : res[j] = sum_t excl*da           (scalar_tensor_tensor + accum_out)

DMA: all loads on the SP HWDGE ring, conf tiles front-loaded so the last
tile's scan is done before its acc arrives (tail = one fused stt pass).
"""

import numpy as np

import concourse.bacc as bacc
import concourse.mybir as mybir
import concourse.tile as tile
from concourse.bass_utils import run_bass_kernel_spmd

B, T = 8192, 1024
N_CORES = 8
ROWS = B // N_CORES  # rows per core
P = 128  # SBUF partitions
NTILES = ROWS // P  # row-tiles per core

# conf-ahead interleave; acc7 lands last (split in two halves so the tail
# chain after the final bytes is only a half-width da+stt)
LOAD_ORDER = [
    ("c", 0), ("a", 0), ("c", 1), ("c", 2), ("a", 1), ("c", 3),
    ("a", 2), ("c", 4), ("a", 3), ("c", 5), ("a", 4), ("c", 6),
    ("a", 5), ("c", 7), ("a", 6), ("a", 7),
]
H = T // 2  # half width for the last tile's tail split

_CACHE = {}


def build_bass():
    f32 = mybir.dt.float32
    Alu = mybir.AluOpType
    nc = bacc.Bacc("TRN2", target_bir_lowering=False, debug=False)
    conf = nc.declare_dram_parameter("confidences", [ROWS, T], f32, isOutput=False)
    acc = nc.declare_dram_parameter("accuracies", [ROWS, T], f32, isOutput=False)
    out = nc.declare_dram_parameter("partials", [P, NTILES + 1], f32, isOutput=True)

    conf_r = conf.rearrange("(n p) t -> n p t", p=P)
    acc_r = acc.rearrange("(n p) t -> n p t", p=P)

    with tile.TileContext(nc) as tc:
        with (
            tc.tile_pool(name="io", bufs=NTILES) as io_pool,
            tc.tile_pool(name="work", bufs=8) as work_pool,
            tc.tile_pool(name="res", bufs=1) as res_pool,
        ):
            res = res_pool.tile([P, NTILES + 1], f32)
            ct, at = {}, {}
            for kind, j in LOAD_ORDER:
                if kind == "c":
                    ct[j] = io_pool.tile([P, T], f32, tag="conf", name=f"conf_t{j}")
                    nc.sync.dma_start(ct[j][:], conf_r[j])
                else:
                    at[j] = io_pool.tile([P, T], f32, tag="acc", name=f"acc_t{j}")
                    if j == NTILES - 1:
                        nc.sync.dma_start(at[j][:, 0:H], acc_r[j][:, 0:H])
                        nc.sync.dma_start(at[j][:, H:T], acc_r[j][:, H:T])
                    else:
                        nc.sync.dma_start(at[j][:], acc_r[j])

            for j in range(NTILES):
                conf_t = ct[j]
                acc_t = at[j]

                # nb = 1 - conf  (ScalarE)
                nb = work_pool.tile([P, T], f32, tag="nb")
                nc.scalar.activation(
                    nb[:],
                    conf_t[:],
                    mybir.ActivationFunctionType.Copy,
                    bias=1.0,
                    scale=-1.0,
                )

                # excl[0]=1, excl[t] = prod_{s<t} nb[s]  (DVE hardware scan)
                excl = work_pool.tile([P, T], f32, tag="excl")
                nc.gpsimd.memset(excl[:, 0:1], 1.0)
                nc.vector.tensor_tensor_scan(
                    excl[:, 1:T],
                    nb[:, 0 : T - 1],
                    nb[:, 0 : T - 1],
                    1.0,
                    Alu.mult,
                    Alu.bypass,
                )

                # da[0] = acc[0]; da[t] = acc[t] - acc[t-1]  (GPSIMD)
                # res[j] = sum_t excl[t]*da[t]  (DVE fused mul + row-sum)
                da = work_pool.tile([P, T], f32, tag="da")
                scr = work_pool.tile([P, T], f32, tag="scr")
                nc.gpsimd.tensor_copy(da[:, 0:1], acc_t[:, 0:1])
                if j == NTILES - 1:
                    # split halves so the tail after the last DMA is short
                    nc.gpsimd.tensor_sub(
                        da[:, 1:H], acc_t[:, 1:H], acc_t[:, 0 : H - 1]
                    )
                    nc.vector.scalar_tensor_tensor(
                        scr[:, 0:H],
                        excl[:, 0:H],
                        1.0,
                        da[:, 0:H],
                        Alu.bypass,
                        Alu.mult,
                        accum_out=res[:, j : j + 1],
                    )
                    nc.gpsimd.tensor_sub(
                        da[:, H:T], acc_t[:, H:T], acc_t[:, H - 1 : T - 1]
                    )
                    nc.vector.scalar_tensor_tensor(
                        scr[:, H:T],
                        excl[:, H:T],
                        1.0,
                        da[:, H:T],
                        Alu.bypass,
                        Alu.mult,
                        accum_out=res[:, j + 1 : j + 2],
                    )
                else:
                    nc.gpsimd.tensor_sub(
                        da[:, 1:T], acc_t[:, 1:T], acc_t[:, 0 : T - 1]
                    )
                    nc.vector.scalar_tensor_tensor(
                        scr[:],
                        excl[:],
                        1.0,
                        da[:],
                        Alu.bypass,
                        Alu.mult,
                        accum_out=res[:, j : j + 1],
                    )

            nc.sync.dma_start(out[:], res[:])
    nc.compile()
    return nc


def make_in_maps(confidences: np.ndarray, accuracies: np.ndarray):
    conf = np.ascontiguousarray(np.asarray(confidences, dtype=np.float32))
    acc = np.ascontiguousarray(np.asarray(accuracies, dtype=np.float32))
    return [
        {
            "confidences": conf[i * ROWS : (i + 1) * ROWS],
            "accuracies": acc[i * ROWS : (i + 1) * ROWS],
        }
        for i in range(N_CORES)
    ]


def reduce_partials(results) -> np.ndarray:
    total = 0.0
    for r in results:
        total += np.sum(r["partials"], dtype=np.float64)
    return np.asarray(-(total / B), dtype=np.float32)


def kernel(confidences: np.ndarray, accuracies: np.ndarray) -> np.ndarray:
    if "nc" not in _CACHE:
        _CACHE["nc"] = build_bass()
    nc = _CACHE["nc"]
    results = run_bass_kernel_spmd(
        nc, make_in_maps(confidences, accuracies), list(range(N_CORES))
    ).results
    return reduce_partials(results)


# revision 12
# speedup vs baseline: 87.8267x; 87.8267x over previous
"""BuzzLoss Trainium2 kernel.

Math (telescoped form of the reference):
    excl[t] = prod_{s<t} (1 - conf[s])          (exclusive cumprod)
    score_b = sum_t excl[b,t] * da[b,t]
    da[b,0] = acc[b,0];  da[b,t] = acc[b,t] - acc[b,t-1]
    out = -mean_b score_b

Derivation: buzz[t] = conf[t]*excl[t] = excl[t] - excl[t+1] telescopes, and
the correction term (1 - sum buzz) * acc[T-1] = cum[T-1]*acc[T-1] cancels
against the boundary of the summation-by-parts.

Sharding: pure data parallel — batch 8192 split across 8 NeuronCores (1024
rows each).  Each core emits per-row scores ([128 x 8] f32); the host sums,
takes the mean, and negates.  No collectives.

Per 128-row tile on-chip:
    ACT   : nb = 1 - conf                   (activation Copy, scale=-1, bias=1)
    GPSIMD: excl[:, 0] = 1                  (memset)
    DVE   : excl[:, 1:] = prefix scan        (tensor_tensor_scan, mult — the
            hardware recurrence computes the whole cumprod in one instruction)
    GPSIMD: da = shifted subtract of acc     (tensor_sub)
    DVE   : res[j] = sum_t excl*da           (scalar_tensor_tensor + accum_out)

DMA: all loads on the SP HWDGE ring, conf tiles front-loaded so the last
tile's scan is done before its acc arrives (tail = one fused stt pass).
"""

import numpy as np

import concourse.bacc as bacc
import concourse.mybir as mybir
import concourse.tile as tile
from concourse.bass_utils import run_bass_kernel_spmd

B, T = 8192, 1024
N_CORES = 8
ROWS = B // N_CORES  # rows per core
P = 128  # SBUF partitions
NTILES = ROWS // P  # row-tiles per core

# conf-ahead interleave; acc7 lands last (split in two halves so the tail
# chain after the final bytes is only a half-width da+stt)
LOAD_ORDER = [
    ("c", 0), ("a", 0), ("c", 1), ("c", 2), ("a", 1), ("c", 3),
    ("a", 2), ("c", 4), ("a", 3), ("c", 5), ("a", 4), ("c", 6),
    ("a", 5), ("c", 7), ("a", 6), ("a", 7),
]
H = T // 2  # half width for the last tile's tail split

_CACHE = {}


def build_bass(reps: int = 1):
    f32 = mybir.dt.float32
    Alu = mybir.AluOpType
    nc = bacc.Bacc("TRN2", target_bir_lowering=False, debug=False)
    conf = nc.declare_dram_parameter("confidences", [ROWS, T], f32, isOutput=False)
    acc = nc.declare_dram_parameter("accuracies", [ROWS, T], f32, isOutput=False)
    out = nc.declare_dram_parameter("partials", [P, NTILES + 1], f32, isOutput=True)

    conf_r = conf.rearrange("(n p) t -> n p t", p=P)
    acc_r = acc.rearrange("(n p) t -> n p t", p=P)

    with tile.TileContext(nc) as tc:
        with (
            tc.tile_pool(name="io", bufs=NTILES) as io_pool,
            tc.tile_pool(name="work", bufs=8) as work_pool,
            tc.tile_pool(name="res", bufs=1) as res_pool,
        ):
            res = res_pool.tile([P, NTILES + 1], f32)
            for rep in range(reps):
                _emit_pipeline(nc, io_pool, work_pool, res, conf_r, acc_r, rep)
            nc.sync.dma_start(out[:], res[:])
    nc.compile()
    return nc


def _emit_pipeline(nc, io_pool, work_pool, res, conf_r, acc_r, rep):
    f32 = mybir.dt.float32
    Alu = mybir.AluOpType
    if True:
        if True:
            ct, at = {}, {}
            for kind, j in LOAD_ORDER:
                if kind == "c":
                    ct[j] = io_pool.tile(
                        [P, T], f32, tag="conf", name=f"conf_t{rep}_{j}"
                    )
                    nc.sync.dma_start(ct[j][:], conf_r[j])
                else:
                    at[j] = io_pool.tile(
                        [P, T], f32, tag="acc", name=f"acc_t{rep}_{j}"
                    )
                    if j == NTILES - 1:
                        nc.sync.dma_start(at[j][:, 0:H], acc_r[j][:, 0:H])
                        nc.sync.dma_start(at[j][:, H:T], acc_r[j][:, H:T])
                    else:
                        nc.sync.dma_start(at[j][:], acc_r[j])

            for j in range(NTILES):
                conf_t = ct[j]
                acc_t = at[j]

                # nb = 1 - conf  (ScalarE)
                nb = work_pool.tile([P, T], f32, tag="nb")
                nc.scalar.activation(
                    nb[:],
                    conf_t[:],
                    mybir.ActivationFunctionType.Copy,
                    bias=1.0,
                    scale=-1.0,
                )

                # excl[0]=1, excl[t] = prod_{s<t} nb[s]  (DVE hardware scan)
                excl = work_pool.tile([P, T], f32, tag="excl")
                nc.gpsimd.memset(excl[:, 0:1], 1.0)
                nc.vector.tensor_tensor_scan(
                    excl[:, 1:T],
                    nb[:, 0 : T - 1],
                    nb[:, 0 : T - 1],
                    1.0,
                    Alu.mult,
                    Alu.bypass,
                )

                # da[0] = acc[0]; da[t] = acc[t] - acc[t-1]  (GPSIMD)
                # res[j] = sum_t excl[t]*da[t]  (DVE fused mul + row-sum)
                da = work_pool.tile([P, T], f32, tag="da")
                scr = work_pool.tile([P, T], f32, tag="scr")
                nc.gpsimd.tensor_copy(da[:, 0:1], acc_t[:, 0:1])
                if j == NTILES - 1:
                    # split halves so the tail after the last DMA is short
                    nc.gpsimd.tensor_sub(
                        da[:, 1:H], acc_t[:, 1:H], acc_t[:, 0 : H - 1]
                    )
                    nc.vector.scalar_tensor_tensor(
                        scr[:, 0:H],
                        excl[:, 0:H],
                        1.0,
                        da[:, 0:H],
                        Alu.bypass,
                        Alu.mult,
                        accum_out=res[:, j : j + 1],
                    )
                    nc.gpsimd.tensor_sub(
                        da[:, H:T], acc_t[:, H:T], acc_t[:, H - 1 : T - 1]
                    )
                    nc.vector.scalar_tensor_tensor(
                        scr[:, H:T],
                        excl[:, H:T],
                        1.0,
                        da[:, H:T],
                        Alu.bypass,
                        Alu.mult,
                        accum_out=res[:, j + 1 : j + 2],
                    )
                else:
                    nc.gpsimd.tensor_sub(
                        da[:, 1:T], acc_t[:, 1:T], acc_t[:, 0 : T - 1]
                    )
                    nc.vector.scalar_tensor_tensor(
                        scr[:],
                        excl[:],
                        1.0,
                        da[:],
                        Alu.bypass,
                        Alu.mult,
                        accum_out=res[:, j : j + 1],
                    )


def make_in_maps(confidences: np.ndarray, accuracies: np.ndarray):
    conf = np.ascontiguousarray(np.asarray(confidences, dtype=np.float32))
    acc = np.ascontiguousarray(np.asarray(accuracies, dtype=np.float32))
    return [
        {
            "confidences": conf[i * ROWS : (i + 1) * ROWS],
            "accuracies": acc[i * ROWS : (i + 1) * ROWS],
        }
        for i in range(N_CORES)
    ]


def reduce_partials(results) -> np.ndarray:
    total = 0.0
    for r in results:
        total += np.sum(r["partials"], dtype=np.float64)
    return np.asarray(-(total / B), dtype=np.float32)


def kernel(confidences: np.ndarray, accuracies: np.ndarray) -> np.ndarray:
    if "nc" not in _CACHE:
        _CACHE["nc"] = build_bass()
    nc = _CACHE["nc"]
    results = run_bass_kernel_spmd(
        nc, make_in_maps(confidences, accuracies), list(range(N_CORES))
    ).results
    return reduce_partials(results)
